# revision 27
# baseline (speedup 1.0000x reference)
"""Localized embedding layer (separable 5x5 Gaussian stencil) on 8 trn2 cores.

Math: out[i,j,:] = sum_{|di|<=2,|dj|<=2} w(di)w(dj) H[i+di,j+dj,:] / den(i,j)
with w(d) = exp(-c*d^2), c = TILE^2/(2 sigma^2), den(i,j) = r(i)*r(j) rank-1.

Per core (32 output grid rows + 2-row halo each side, zero padded), inputs
staged host-side as bf16 (tolerance 2e-2):
  - row tiles are FLAT [128, 1024] bf16 (3D APs defeat the DVE 2x packed
    mode); i-conv pair sums t1=a1+a3, t2=a0+a4 BOTH on DVE (GpSimd shares
    DVE's SBUF ports - concurrent GpSimd tensor ops halve DVE throughput)
  - i-conv combine + j-conv on TensorE: per half, THREE bf16 matmuls
    accumulated in PSUM: B@a2 + (w1*B)@t1 + (w2*B)@t2, with B the UNIFORM
    symmetric band w(|dj|)/W_full (same weights both halves). The 8 output
    columns j in {0,1,126..129,254,255} where the uniform band is wrong
    (half-boundary crossing / grid edge) are recomputed by a strip pass
    that is ALSO pure matmuls: psf[i,jo] accumulates (w(jo-jin)/r(jo) *
    Wstrip) @ xs[:,jin,:] over the <=5 j-taps, where Wstrip is the [36,32]
    i-conv matrix (contraction dim = grid row).
  - ScalarE: one PSUM->SBUF f32 copy per row with per-row scale W_full/r(i)
  - DMA: loads on the gpsimd SWDGE ring; stores alternate between the sync
    and scalar HWDGE rings (per-ring transfers are FIFO; rings run in
    parallel)
"""

import sys
import numpy as np
import ml_dtypes

if "/opt/trn_rl_repo" not in sys.path:
    sys.path.insert(0, "/opt/trn_rl_repo")

BF16 = ml_dtypes.bfloat16

G = 256          # grid side
D = 512          # feature dim
P = 2            # grid_step halo
NC = 8           # cores
RPC = G // NC    # rows per core = 32
TILE = 448.0
SIGMA = 200.0

# strip columns: grid edges + half-boundary neighborhood
JO_LIST = [0, 1, 126, 127, 128, 129, 254, 255]
# xs slot layout: jin columns the strip needs
XS_COLS = list(range(0, 4)) + list(range(124, 132)) + list(range(252, 256))
XS_SLOT = {j: s for s, j in enumerate(XS_COLS)}

_cache = {}


def _weights():
    c = TILE * TILE / (2.0 * SIGMA * SIGMA)
    return np.exp(-c * np.arange(-P, P + 1) ** 2)   # [w2,w1,1,w1,w2] f64


def _r_vec():
    """r(i) = sum of valid 1D taps at row i (same for columns)."""
    w = _weights()
    r = np.zeros(G)
    for d in range(-P, P + 1):
        lo, hi = max(0, -d), min(G, G - d)
        r[lo:hi] += w[d + P]
    return r


def _strip_taps():
    """[(jo, [(xs_slot, w_scale), ...]), ...] for the 8 strip columns."""
    w = _weights()
    r = _r_vec()
    out = []
    for jo in JO_LIST:
        taps = []
        for dj in range(-P, P + 1):
            jin = jo + dj
            if 0 <= jin < G:
                taps.append((XS_SLOT[jin], w[dj + P] / r[jo]))
        out.append((jo, taps))
    return out


def _host_consts():
    w = _weights()
    r = _r_vec()
    w_full = w.sum()
    w1, w2 = w[1], w[0]
    # uniform symmetric band B[jout, jin] = w(|jout-jin|)/W_full, 128x128
    Bu = np.zeros((128, 128))
    for d in range(-P, P + 1):
        for jout in range(128):
            jin = jout + d
            if 0 <= jin < 128:
                Bu[jout, jin] = w[d + P] / w_full
    wmat = np.zeros((128, 3, 128), dtype=BF16)
    wmat[:, 0, :] = Bu.T.astype(BF16)
    wmat[:, 1, :] = (w1 * Bu.T).astype(BF16)
    wmat[:, 2, :] = (w2 * Bu.T).astype(BF16)
    # strip i-conv lhsT [36, 32]: Tstrip[i] = sum_k w[k] * XS[i+k]
    wstrip = np.zeros((RPC + 2 * P, RPC))
    for i in range(RPC):
        for k in range(5):
            wstrip[i + k, i] = w[k]
    # scaled copies, one per (jo, tap) pair
    taps = _strip_taps()
    ntap = sum(len(t) for _, t in taps)
    ws = np.zeros((RPC + 2 * P, ntap, RPC), dtype=BF16)
    idx = 0
    tap_index = []     # per jo: [(slot, ws_idx), ...]
    for jo, tl in taps:
        entry = []
        for slot, scale in tl:
            ws[:, idx, :] = (scale * wstrip).astype(BF16)
            entry.append((slot, idx))
            idx += 1
        tap_index.append(entry)
    # per-core scales: 1/r_row(i) (the 1/w_full j-normalizer is in Bu)
    scales, sfixes = [], []
    for c in range(NC):
        s = (1.0 / r[RPC * c: RPC * (c + 1)]).astype(np.float32)
        scales.append(np.broadcast_to(s[None, :], (128, RPC)).copy())
        # strip scale: 1 / r_row(i) (j-normalizer folded into ws)
        sf = np.zeros((128, 1), dtype=np.float32)
        sf[:RPC, 0] = (1.0 / r[RPC * c: RPC * (c + 1)]).astype(np.float32)
        sfixes.append(sf)
    return wmat, ws, tap_index, scales, sfixes


def _build_nc():
    import concourse.bass as bass
    import concourse.mybir as mybir
    import concourse.tile as tile
    from concourse import bacc

    f32 = mybir.dt.float32
    bf16 = mybir.dt.bfloat16
    add = mybir.AluOpType.add

    NR = RPC + 2 * P
    _, _, tap_index, _, _ = _get_consts()
    ntap = sum(len(t) for t in tap_index)

    nc = bacc.Bacc(None, target_bir_lowering=False, debug=False)
    x_dram = nc.declare_dram_parameter("x", [NR // 2, 128, 2, 2, D], bf16, isOutput=False)
    xst_dram = nc.declare_dram_parameter("xstrip", [NR, 16, D], bf16, isOutput=False)
    wm_dram = nc.declare_dram_parameter("wmat", [128, 3, 128], bf16, isOutput=False)
    wf_dram = nc.declare_dram_parameter("wstrip", [NR, ntap, RPC], bf16, isOutput=False)
    sc_dram = nc.declare_dram_parameter("scale", [128, RPC], f32, isOutput=False)
    sf_dram = nc.declare_dram_parameter("sfix", [128, 1], f32, isOutput=False)
    y_dram = nc.declare_dram_parameter("y", [RPC // 2, 128, 2, 2, D], bf16, isOutput=True)
    yf_dram = nc.declare_dram_parameter("yfix", [RPC, 8, D], bf16, isOutput=True)

    NXS = len(XS_COLS)

    with tile.TileContext(nc) as tc:
        with (
            tc.tile_pool(name="const", bufs=1) as cpool,
            tc.tile_pool(name="x", bufs=NR // 2) as xpool,
            tc.tile_pool(name="tmp", bufs=4) as tpool,
            tc.tile_pool(name="out", bufs=16) as opool,
            tc.tile_pool(name="fix", bufs=1) as fpool,
            tc.tile_pool(name="psum", bufs=3, space="PSUM") as ppool,
            tc.tile_pool(name="psfix", bufs=1, space="PSUM") as pfpool,
        ):
            wt = cpool.tile([128, 3, 128], bf16)
            nc.sync.dma_start(wt[:], wm_dram[:])
            wft = cpool.tile([NR, ntap, RPC], bf16)
            nc.sync.dma_start(wft[:], wf_dram[:])
            st = cpool.tile([128, RPC], f32)
            nc.sync.dma_start(st[:], sc_dram[:])
            sft = cpool.tile([128, 1], f32)
            nc.sync.dma_start(sft[:], sf_dram[:])

            # strip input FIRST (strip matmuls are interleaved into early
            # rows; PE executes in order, so xs must land before them):
            # xs[r, slot, d] = x[r, XS_COLS[slot], d]
            xs = fpool.tile([NR, NXS, D], bf16, tag="xs")
            nc.sync.dma_start(xs[:], xst_dram[:])

            # prefetch ALL input rows up front, two rows per DMA
            # (x layout [pair, p, r, h, D]: per-partition 4KB contiguous)
            xt = {}
            for m in range(NR // 2):
                t = xpool.tile([128, 2048], bf16, tag="xrow", name=f"xr{m}")
                nc.sync.dma_start(t[:], x_dram[m].rearrange("p r h d -> p (r h d)"))
                xt[2 * m] = t[:, 0:1024]
                xt[2 * m + 1] = t[:, 1024:2048]

            # strip pass p handles JO_LIST[2p], JO_LIST[2p+1]
            def emit_strip_pass(p):
                psf = pfpool.tile([RPC, 2, D], f32, tag="psf")
                # interleave the two columns' accumulation chains so
                # consecutive matmuls hit different PSUM banks
                ent = [tap_index[2 * p], tap_index[2 * p + 1]]
                order = []
                for k in range(max(len(ent[0]), len(ent[1]))):
                    for c in range(2):
                        if k < len(ent[c]):
                            order.append((c, k))
                for c, k in order:
                    slot, widx = ent[c][k]
                    nc.tensor.matmul(
                        psf[:, c, :], wft[:, widx, :], xs[:, slot, :],
                        start=(k == 0), stop=(k == len(ent[c]) - 1),
                    )
                fs = fpool.tile([RPC, 2, D], bf16, tag=f"fs{p}")
                nc.scalar.mul(fs[:], psf[:], sft[0:RPC, 0:1])
                nc.sync.dma_start(yf_dram[:, 2 * p:2 * p + 2, :], fs[:])

            # strip passes run first: they fill the PE while the row
            # loads stream in (xs is the first load on the ring)
            for p in range(4):
                emit_strip_pass(p)

            # ---- main loop: row pairs ----
            # obw layout [128, h, r, D]: per-partition offset h*1024 + r*512,
            # so the per-half store source is a FLAT [124, 1024] slice
            # covering both rows of the pair.
            for pair in range(RPC // 2):
                obw = opool.tile([128, 2, 2, D], bf16, tag="obw")  # [p, r, h, D]
                for r in range(2):
                    i = 2 * pair + r
                    a0, a1, a2, a3, a4 = (xt[i + k] for k in range(5))
                    t1 = tpool.tile([128, 1024], bf16, tag="t1")
                    nc.vector.tensor_tensor(t1[:], a1, a3, add)
                    t2 = tpool.tile([128, 1024], bf16, tag="t2")
                    nc.vector.tensor_tensor(t2[:], a0, a4, add)
                    ps = ppool.tile([128, 2, D], f32, tag="ps")
                    # tap-major order: consecutive matmuls hit different
                    # PSUM banks, so they pipeline (~216ns/MM) instead of
                    # serializing on the accumulation RAW (~380ns/MM)
                    for tap, rhs in ((0, a2), (1, t1), (2, t2)):
                        for hm in range(2):
                            sl = slice(512 * hm, 512 * hm + 512)
                            nc.tensor.matmul(
                                ps[:, hm, :], wt[:, tap, :], rhs[:, sl],
                                start=(tap == 0), stop=(tap == 2),
                            )
                    nc.scalar.mul(obw[:, r, :, :], ps[:], st[:, i:i + 1])
                nc.gpsimd.dma_start(y_dram[pair, 2:126, :, :, :], obw[2:126, :, :, :])

    nc.finalize()
    return nc


def _get_consts():
    if "consts" not in _cache:
        _cache["consts"] = _host_consts()
    return _cache["consts"]


def _get_program():
    if "nc" not in _cache:
        _cache["nc"] = _build_nc()
    return _cache["nc"], _get_consts()


def _in_maps(H):
    wmat, ws, tap_index, scales, sfixes = _get_consts()
    H3 = H.reshape(G, G, D)
    Hp = np.zeros((G + 2 * P, G, D), dtype=BF16)
    Hp[P:P + G] = H3.astype(BF16)
    NR = RPC + 2 * P
    in_maps = []
    for c in range(NC):
        sh = Hp[RPC * c: RPC * c + NR]
        # device x layout [pair, p, r, h, D]; row (2*pair+r), col j = h*128+p
        shard = np.ascontiguousarray(
            sh.reshape(NR // 2, 2, 2, 128, D).transpose(0, 3, 1, 2, 4)
        )
        xstrip = np.ascontiguousarray(sh[:, XS_COLS, :])
        in_maps.append(
            {"x": shard, "xstrip": xstrip, "wmat": wmat, "wstrip": ws,
             "scale": scales[c], "sfix": sfixes[c]}
        )
    return in_maps


def _unshard(res):
    # device y layout [pair, p, r, h, D] -> row i = 2*pair+r, col j = h*128+p
    outs = []
    for c in range(NC):
        y = res[c]["y"].transpose(0, 2, 3, 1, 4).reshape(RPC, G, D)
        yf = res[c]["yfix"]
        y = np.ascontiguousarray(y)
        for k, jo in enumerate(JO_LIST):
            y[:, jo, :] = yf[:, k, :]
        outs.append(y.reshape(RPC * G, D))
    return np.concatenate(outs, axis=0).astype(np.float32)


def kernel(H, xy=None):
    from concourse.bass_utils import run_bass_kernel_spmd

    nc, _ = _get_program()
    res = run_bass_kernel_spmd(nc, _in_maps(H), list(range(NC))).results
    return _unshard(res)
